# revision 25
# baseline (speedup 1.0000x reference)
"""Causal self-attention (GQA + RoPE) Trainium2 Bass kernel.

Problem: B=2, T=2048, C=2048, H=16 q-heads, HK=4 kv-heads, HD=128.
Sharding: 8 cores = (batch b in {0,1}) x (kv-head group g in {0..3}).
Each core computes its batch's 4 q-heads / 1 kv-head slice end-to-end
(QKV proj -> RoPE -> causal attention -> o-proj partial), returning a
[T, C] partial y; the host sums the 4 group partials per batch.

On-device layout notes:
 - Contractions run on the PE; all operands need the contraction dim on
   the SBUF partition axis, so x is DMA-transposed (xbar) to xT chunks.
 - Scores are computed transposed (ST[tk, tq]) so that softmax
   normalization sums (over tk) run on the PE via an all-ones lhsT, and
   att@v needs no transposes at all.
 - Causal structure: tk blocks past the diagonal are skipped; the four
   partial blocks per (head, tq-chunk) compute only valid columns, with
   a [128,128] additive -1e30 mask on the diagonal sub-block.
 - bf16 everywhere on the PE (1 cyc/row, FWL weight loads), fp32 PSUM
   accumulation, fp32 softmax statistics.
 - Every logically-separate chunk lives in its own tile: Tile tracks
   dependencies per tile, so shared mega-tiles serialize phases.
"""
import contextlib

import numpy as np
import ml_dtypes

import concourse.bass as bass
import concourse.tile as tile
import concourse.mybir as mybir
from concourse.bass_utils import run_bass_kernel_spmd

BF16 = ml_dtypes.bfloat16

B, T, C = 2, 2048, 2048
H, HK, HD = 16, 4, 128
GQ = H // HK            # q heads per core = 4
NCORES = 8
TQC = 512               # tq chunk width
NTQ = T // TQC          # 4
NKC = C // 128          # 16 contraction chunks
NTK = T // 128          # 16 tk blocks
SCALE = 1.0 / float(np.sqrt(HD))
MASKVAL = -1.0e30

DT = mybir.dt.bfloat16
F32 = mybir.dt.float32


def _split_waits(nc, maxw=1):
    """This walrus build rejects instructions with >1 sync wait; move
    overflow waits onto same-engine nops inserted just before."""
    cnt = 0
    for f in nc.m.functions:
        for bb in f.blocks:
            idx = 0
            while idx < len(bb.instructions):
                inst = bb.instructions[idx]
                si = inst.sync_info
                waits = list(si.on_wait) if si is not None and si.on_wait else []
                if len(waits) > maxw:
                    updates = list(si.on_update) if si.on_update else []
                    keep, rest = waits[:maxw], waits[maxw:]
                    pos = idx
                    while rest:
                        chunk, rest = rest[:maxw], rest[maxw:]
                        cnt += 1
                        nop = mybir.InstNoOp(
                            name=f"waitsplit_{cnt}", engine=inst.engine,
                            ins=[], outs=[])
                        nop.sync_info = mybir.SyncInfo(on_wait=chunk, on_update=[])
                        nc.register_instruction(nop, overwrite=True)
                        bb.instructions.insert(pos, nop)
                        pos += 1
                        idx += 1
                    inst.sync_info = mybir.SyncInfo(on_wait=keep, on_update=updates)
                idx += 1
    return cnt


def build(reps: int = 1):
    nc = bass.Bass(target_bir_lowering=False)
    xTd = nc.dram_tensor("xT", [C, T], DT, kind="ExternalInput")
    cosT = nc.dram_tensor("cosT", [HD, T], DT, kind="ExternalInput")
    sinT = nc.dram_tensor("sinT", [HD, T], DT, kind="ExternalInput")
    wq = nc.dram_tensor("wq", [C, GQ * HD], DT, kind="ExternalInput")
    wk = nc.dram_tensor("wk", [C, HD], DT, kind="ExternalInput")
    wv = nc.dram_tensor("wv", [C, HD], DT, kind="ExternalInput")
    wo = nc.dram_tensor("wo", [GQ * HD, C], DT, kind="ExternalInput")
    bqT = nc.dram_tensor("bqT", [HD, GQ], F32, kind="ExternalInput")
    bkT = nc.dram_tensor("bkT", [HD, 1], F32, kind="ExternalInput")
    bvr = nc.dram_tensor("bvr", [1, HD], F32, kind="ExternalInput")
    yp = nc.dram_tensor("yp", [T, C], DT, kind="ExternalOutput")
    rcscr = nc.dram_tensor("rcscr", [GQ * NTQ, TQC], F32)

    with tile.TileContext(nc) as tc, contextlib.ExitStack() as ctx:
        const = ctx.enter_context(tc.tile_pool(name="const", bufs=1))
        xtp = ctx.enter_context(tc.tile_pool(name="xtp", bufs=1))
        resid = ctx.enter_context(tc.tile_pool(name="resid", bufs=1))
        ytnp = ctx.enter_context(tc.tile_pool(name="ytnp", bufs=1))
        stage = ctx.enter_context(tc.tile_pool(name="stage", bufs=4))
        nrm = ctx.enter_context(tc.tile_pool(name="nrm", bufs=2))
        est = ctx.enter_context(tc.tile_pool(name="est", bufs=6))
        outp = ctx.enter_context(tc.tile_pool(name="outp", bufs=3))
        ps_proj = ctx.enter_context(tc.tile_pool(name="ps_proj", bufs=2, space="PSUM"))
        ps_sc = ctx.enter_context(tc.tile_pool(name="ps_sc", bufs=2, space="PSUM"))
        ps_y = ctx.enter_context(tc.tile_pool(name="ps_y", bufs=2, space="PSUM"))
        ps_sum = ctx.enter_context(tc.tile_pool(name="ps_sum", bufs=2, space="PSUM"))

        # ---- constants / weights to SBUF (ACT hwdge queue), in the order
        # the projection phases consume them: wk, biases/cos/sin, wv, wq, wo
        wk_all = const.tile([128, NKC, HD], DT)
        nc.scalar.dma_start(
            out=wk_all, in_=wk[:, :].rearrange("(k p) m -> p k m", p=128))
        wk_t = [wk_all[:, kc, :] for kc in range(NKC)]
        bq_sb = const.tile([HD, GQ], F32)
        bk_sb = const.tile([HD, 1], F32)
        bvb_sb = const.tile([128, HD], F32)
        nc.scalar.dma_start(out=bq_sb, in_=bqT[:, :])
        nc.scalar.dma_start(out=bk_sb, in_=bkT[:, :])
        nc.scalar.dma_start(out=bvb_sb, in_=bass.AP(bvr, 0, [[0, 128], [1, HD]]))
        cos_sb = const.tile([HD, T], DT)
        sin_sb = const.tile([HD, T], DT)
        nc.scalar.dma_start(out=cos_sb, in_=cosT[:, :])
        nc.scalar.dma_start(out=sin_sb, in_=sinT[:, :])
        wv_all = const.tile([128, NKC, HD], DT)
        wq_all = const.tile([128, NKC, GQ * HD], DT)
        wo_all = const.tile([HD, GQ, C], DT)
        wv_t = [wv_all[:, kc, :] for kc in range(NKC)]
        wq_t = [wq_all[:, kc, :] for kc in range(NKC)]
        wo_t = [wo_all[:, h, :] for h in range(GQ)]
        ones_sb = const.tile([128, 1], DT)
        nc.vector.memset(ones_sb, 1.0)
        # causal mask for ST blocks: keep where col >= row, else -1e30
        mask_sb = const.tile([128, 128], F32)
        nc.gpsimd.memset(mask_sb, 0.0)
        nc.gpsimd.affine_select(
            out=mask_sb, in_=mask_sb,
            compare_op=mybir.AluOpType.is_ge, fill=MASKVAL,
            base=0, pattern=[[1, 128]], channel_multiplier=-1)

        # per-chunk resident tiles
        xtp_t = [xtp.tile([128, 2, T], DT, tag=f"xtp{p}", name=f"xtp{p}")
                 for p in range(NKC // 2)]
        xt = [xtp_t[kc // 2][:, kc % 2, :] for kc in range(NKC)]
        qTt = [[resid.tile([HD, TQC], DT, tag=f"qT{h}_{j}", name=f"qT{h}_{j}")
                for j in range(NTQ)] for h in range(GQ)]
        kTt = [resid.tile([HD, TQC], DT, tag=f"kT{j}", name=f"kT{j}")
               for j in range(NTQ)]
        vt = [resid.tile([128, HD], DT, tag=f"v{i}", name=f"v{i}")
              for i in range(NTK)]

        def rope_store(psum_src, bias_ap, dst_ap, j0):
            """dst = rope(psum_src + bias).

            sin_sb holds the half-swapped, sign-folded sin (host-prepped:
            rows 0:64 = sin[64:128], rows 64:128 = -sin[0:64]), so
            rot_half reduces to a full-width multiply followed by a
            partition half-swap done with two SBUF->SBUF DMAs."""
            qs = stage.tile([128, TQC], DT, tag="qs")
            nc.vector.tensor_scalar(
                out=qs, in0=psum_src, scalar1=bias_ap, scalar2=None,
                op0=mybir.AluOpType.add)
            tmp = stage.tile([128, TQC], DT, tag="tmp")
            nc.vector.tensor_mul(tmp, qs, cos_sb[:, j0:j0 + TQC])
            prod = stage.tile([128, TQC], DT, tag="prod")
            nc.vector.tensor_mul(prod, qs, sin_sb[:, j0:j0 + TQC])
            prodsw = stage.tile([128, TQC], DT, tag="prodsw")
            nc.sync.dma_start(out=prodsw[0:64, :], in_=prod[64:128, :])
            nc.sync.dma_start(out=prodsw[64:128, :], in_=prod[0:64, :])
            nc.vector.tensor_add(dst_ap, tmp, prodsw)

        for rep in range(reps):
            # ---- load xT chunk pairs, alternating across hwdge queues ----
            for p in range(NKC // 2):
                eng = nc.sync if p % 2 == 0 else nc.scalar
                eng.dma_start(
                    out=xtp_t[p],
                    in_=xTd[p * 256:(p + 1) * 256, :].rearrange(
                        "(i p) t -> p i t", p=128))
            if rep == 0:
                # bulk weights go behind the xT chunks in the ACT queue:
                # wv is needed first, wq next, wo only for the o-proj tail
                nc.scalar.dma_start(
                    out=wv_all,
                    in_=wv[:, :].rearrange("(k p) m -> p k m", p=128))
                nc.scalar.dma_start(
                    out=wq_all,
                    in_=wq[:, :].rearrange("(k p) m -> p k m", p=128))
                nc.scalar.dma_start(
                    out=wo_all,
                    in_=wo[:, :].rearrange("(h p) m -> p h m", p=128))

            # ---- kT + v projections, round-robin over all four PSUM
            # pools (tags shared with their later attention users) so many
            # accumulation groups ride out the xT DMA stream ----
            pools = [(ps_proj, "proj"), (ps_sc, "sc"), (ps_y, "py"),
                     (ps_sum, "psum")]

            # wave 1: all 4 k-chunks + v blocks 0..3, kc-major so the PE
            # consumes each xT chunk as its DMA lands (8 live PSUM groups)
            pk_t = []
            for j in range(NTQ):
                pool, ptag = pools[j % 4]
                pk_t.append(pool.tile([128, TQC], F32, tag=ptag,
                                      name=f"pk{j}"))
            pv_t = []
            for tk in range(4):
                pool, ptag = pools[tk % 4]
                pv_t.append(pool.tile([128, TQC], F32, tag=ptag,
                                      name=f"pv{tk}"))
            for kc in range(NKC):
                for j in range(NTQ):
                    nc.tensor.matmul(
                        pk_t[j], wk_t[kc], xt[kc][:, j * TQC:(j + 1) * TQC],
                        start=(kc == 0), stop=(kc == NKC - 1))
                for tk in range(4):
                    nc.tensor.matmul(
                        pv_t[tk][:, 0:HD], xt[kc][:, tk * 128:tk * 128 + 128],
                        wv_t[kc], start=(kc == 0), stop=(kc == NKC - 1))
            for j in range(NTQ):
                rope_store(pk_t[j], bk_sb[:, 0:1], kTt[j], j * TQC)
            for tk in range(4):
                nc.vector.tensor_add(vt[tk], pv_t[tk][:, 0:HD], bvb_sb)

            # remaining v blocks (xT fully resident by now)
            for tk in range(4, NTK):
                t0 = tk * 128
                pool, ptag = pools[tk % 4]
                pv = pool.tile([128, TQC], F32, tag=ptag, name=f"pv{tk}")
                for kc in range(NKC):
                    nc.tensor.matmul(
                        pv[:, 0:HD], xt[kc][:, t0:t0 + 128], wv_t[kc],
                        start=(kc == 0), stop=(kc == NKC - 1))
                nc.vector.tensor_add(vt[tk], pv[:, 0:HD], bvb_sb)

            # ---- per tq chunk: q proj (PE) interleaves with attention
            # (fills the PE while ACT works the exp stream); o-proj runs
            # as a dense PE pass at the end.
            ytn_all = {}
            for j in range(NTQ):
                ytn = []
                for h in range(GQ):
                    j0 = j * TQC
                    pq = ps_proj.tile([128, TQC], F32, tag="proj")
                    for kc in range(NKC):
                        nc.tensor.matmul(
                            pq, wq_t[kc][:, h * HD:(h + 1) * HD],
                            xt[kc][:, j0:j0 + TQC],
                            start=(kc == 0), stop=(kc == NKC - 1))
                    rope_store(pq, bq_sb[:, h:h + 1], qTt[h][j], j0)
                    py = ps_y.tile([HD, TQC], F32, tag="py")
                    psum = ps_sum.tile([1, TQC], F32, tag="psum")
                    nblk = 4 * j + 4
                    for i in range(nblk):
                        s = i - 4 * j  # >=0 for partial blocks
                        c0 = 128 * s if s > 0 else 0
                        jk, ik = divmod(i, 4)
                        sc = ps_sc.tile([128, TQC], F32, tag="sc")
                        nc.tensor.matmul(
                            sc[:, c0:TQC], kTt[jk][:, ik * 128:(ik + 1) * 128],
                            qTt[h][j][:, c0:TQC], start=True, stop=True)
                        if s >= 0:
                            # mask diagonal sub-block [128, 128] at cols c0
                            nc.vector.tensor_add(
                                sc[:, c0:c0 + 128], sc[:, c0:c0 + 128], mask_sb)
                        ex = est.tile([128, TQC], DT)
                        if s > 0:
                            nc.gpsimd.memset(ex[:, 0:c0], 0.0)
                        nc.scalar.activation(
                            out=ex[:, c0:TQC], in_=sc[:, c0:TQC],
                            func=mybir.ActivationFunctionType.Exp, scale=SCALE)
                        nc.tensor.matmul(
                            py[:, c0:TQC], vt[i], ex[:, c0:TQC],
                            start=(i == 0), stop=(i == nblk - 1))
                        nc.tensor.matmul(
                            psum[:, c0:TQC], ones_sb, ex[:, c0:TQC],
                            start=(i == 0), stop=(i == nblk - 1))
                    # normalize: yT[d, tq] / sum[tq]
                    rc = nrm.tile([1, TQC], F32, tag="rc")
                    nc.vector.reciprocal(out=rc, in_=psum)
                    rcb = nrm.tile([HD, TQC], F32, tag="rcb")
                    idx = h * NTQ + j
                    nc.scalar.dma_start(out=rcscr[idx:idx + 1, :], in_=rc)
                    nc.scalar.dma_start(
                        out=rcb, in_=bass.AP(rcscr, idx * TQC, [[0, HD], [1, TQC]]))
                    yt = ytnp.tile([HD, TQC], DT, tag=f"yt{h}_{j}",
                                   name=f"yt{h}_{j}")
                    nc.vector.tensor_mul(yt, py, rcb)
                    ytn.append(yt)
                ytn_all[j] = ytn

            # ---- o-proj: y[tq, :] = sum_h yT_h.T @ Wo_h ----
            for j in range(NTQ):
                ytn = ytn_all[j]
                for t in range(4):  # four 128-row q tiles in this chunk
                    trow = j * TQC + t * 128
                    ot = outp.tile([128, C], DT)
                    for cc in range(4):
                        c0 = cc * TQC
                        po = ps_proj.tile([128, TQC], F32, tag="proj")
                        for h in range(GQ):
                            nc.tensor.matmul(
                                po, ytn[h][:, t * 128:(t + 1) * 128],
                                wo_t[h][:, c0:c0 + TQC],
                                start=(h == 0), stop=(h == GQ - 1))
                        if (t + cc) % 2 == 0:
                            nc.scalar.copy(out=ot[:, c0:c0 + TQC], in_=po)
                        else:
                            nc.vector.tensor_copy(out=ot[:, c0:c0 + TQC], in_=po)
                    oeng = nc.sync if t % 2 == 0 else nc.scalar
                    oeng.dma_start(out=yp[trow:trow + 128, :], in_=ot)
    _split_waits(nc, maxw=1)
    return nc


def _in_maps(x, cos, sin, Wq, bq, Wk, bk, Wv, bv, Wo):
    maps = []
    for c in range(NCORES):
        b, g = divmod(c, HK)
        qsl = slice(g * GQ * HD, (g + 1) * GQ * HD)
        ksl = slice(g * HD, (g + 1) * HD)
        maps.append({
            "xT": np.ascontiguousarray(x[b].T.astype(BF16)),
            "cosT": np.ascontiguousarray(cos[b].T.astype(BF16)),
            "sinT": np.ascontiguousarray(np.concatenate(
                [sin[b].T[64:128], -sin[b].T[0:64]], axis=0).astype(BF16)),
            "wq": np.ascontiguousarray(Wq[:, qsl].astype(BF16)),
            "wk": np.ascontiguousarray(Wk[:, ksl].astype(BF16)),
            "wv": np.ascontiguousarray(Wv[:, ksl].astype(BF16)),
            "wo": np.ascontiguousarray(Wo[qsl, :].astype(BF16)),
            "bqT": np.ascontiguousarray(
                bq[qsl].reshape(GQ, HD).T.astype(np.float32)),
            "bkT": np.ascontiguousarray(
                bk[ksl].reshape(HD, 1).astype(np.float32)),
            "bvr": np.ascontiguousarray(
                bv[ksl].reshape(1, HD).astype(np.float32)),
        })
    return maps


_nc_cache = {}


def kernel(x, cos, sin, Wq, bq, Wk, bk, Wv, bv, Wo):
    x, cos, sin = np.asarray(x), np.asarray(cos), np.asarray(sin)
    Wq, bq = np.asarray(Wq), np.asarray(bq)
    Wk, bk = np.asarray(Wk), np.asarray(bk)
    Wv, bv = np.asarray(Wv), np.asarray(bv)
    Wo = np.asarray(Wo)
    if "nc" not in _nc_cache:
        _nc_cache["nc"] = build(reps=1)
    nc = _nc_cache["nc"]
    maps = _in_maps(x, cos, sin, Wq, bq, Wk, bk, Wv, bv, Wo)
    res = run_bass_kernel_spmd(nc, maps, core_ids=list(range(NCORES)))
    out = np.zeros((B, T, C), dtype=np.float32)
    for c in range(NCORES):
        b = c // HK
        out[b] += res.results[c]["yp"].astype(np.float32)
    return out
